# revision 5
# baseline (speedup 1.0000x reference)
"""Trainium2 Bass kernel for the ConvolutionalOverlap problem.

Reference computation (x: [2, 1, 256, 256] f32, w1/w2 scalar):
    out[b, i, h, w] = w1 * x[b, 0, h, w - (i+1)//2] + w2 * x[b, 0, h, w + (i+2)//2]
    (terms outside [0, W) are zero), out shape [2, 256, 256, 256].

Strategy (pure SPMD across 8 cores, identical program, different data):
  - Flatten (b, h) into 512 rows; shard 64 rows per core.  Each core's 64
    rows are duplicated onto both SBUF partition halves: partitions 0..63
    compute output columns w in [0, 128), partitions 64..127 compute
    w in [128, 256), so one free-dim access pattern serves all 128
    partitions and zero padding implements the boundary masks.
  - fp16 end-to-end on device.  The kernel is HBM-WRITE-bound (16 MB f32
    output per core); fp16 halves that to 8 MB (rel err ~7e-4, far inside
    the 2e-2 gate) AND unlocks the DVE 2x_1p perf mode (2 elem/cycle/lane),
    which fp32 tensor_tensor cannot use.
  - The host pre-stages A = fp16(w1*x) and B = fp16(w2*x) (plus one-element-
    shifted copies A1/B1 so every DVE access stays 4-byte aligned — a 2x_1p
    requirement).  Host staging is ~0.5 MB total vs the 64 MB output, and it
    lets the device run pure tensor_tensor ADDs: scalar_tensor_tensor has no
    2x uop (cost model: bf16 STT = 1x), so folding w1 on-device would halve
    DVE throughput.
  - Channel decomposition: ch = 4t + r.  s1 = (ch+1)//2 and s2 = (ch+2)//2
    are affine in t for fixed r in {0,1,2,3} with t-stride 2 elements
    (4 bytes — aligned).  One DVE tensor_tensor per (group, r):
      r=0: A [128-2t+w"]  + B1[1024+2t+w"]   (s1=2t,   s2=2t+1)
      r=1: A1[382-2t+w"]  + B1[1024+2t+w"]   (s1=2t+1, s2=2t+1)
      r=2: A1[382-2t+w"]  + B [642+2t+w"]    (s1=2t+1, s2=2t+2)
      r=3: A [126-2t+w"]  + B [642+2t+w"]    (s1=2t+2, s2=2t+2)
    (offsets into the packed [128, 1280] fp16 input; segments
     A@0, A1@256, B@512, B1@896.)
  - DVE instructions are chunked COARSER than the 6 output-DMA groups
    (single shared O tile; chunks [16,80,64,96] channels x 4 phases = 16
    instructions): on HW every DVE op costs ~360 cycles of issue+DRAIN on
    top of streaming (24 ops measured 26.2 us vs the 18.5 us stream model),
    so fewer/larger ops buy back ~3 us while the graduated DMA groups
    [16,32,48,64,48,48] keep the first output DMA launching ~2.3 us in.
    Each group's DMA waits on the chunk counter covering its channels;
    out-DMAs alternate between the two HWDGE rings (SP/ACT).
    1 in-DMA + 6 out-DMAs stays <= 8 DMAHW sem lanes.

Per core: ~0.3 MB in, 8.4 MB out.  DVE: 16 TT ops at 2x ~= 16384 stream +
16*~360 overhead cycles at 0.96 GHz ~= 23 us; out-DMA 8.39 MB at the
~341-358 GB/s/core HBM write limit ~= 23.4-24.6 us -> balanced at the
write roofline, ~2x the f32 variant (49.2 us measured).
"""

import sys

import numpy as np

if "/opt/trn_rl_repo" not in sys.path:
    sys.path.insert(0, "/opt/trn_rl_repo")

import concourse.bass as bass
import concourse.mybir as mybir
from concourse.ap import AP

F16 = mybir.dt.float16
P = 128          # SBUF partitions
W = 256          # spatial width == number of output channels
WH = W // 2      # output columns per partition half
XW = 1280        # packed input width: A[0:256) A1[256:512) B[512:896) B1[896:1280)
ROWS = 512       # B * H
NCORES = 8
RPC = ROWS // NCORES  # rows per core (64)
# Channel group sizes (sum 256, all multiples of 4).  Graduated so the
# first output DMA launches early while later, larger groups keep the DMA
# queue fed.  1 in-DMA + len(GROUPS) out-DMAs must stay <= 8 DMAHW sem
# lanes (a 9th DMA wraps onto lane 0, adding a 2nd sync-wait this walrus
# codegen path rejects).
GROUPS = [16, 32, 48, 64, 48, 48]
# DVE work is chunked COARSER than the DMA groups: every DVE instruction
# pays a ~360-cycle issue+DRAIN overhead on HW (measured: 24 instrs ->
# 26.2 us vs the 18.5 us streaming model), so chunks merge groups
# [g0, g1+g2, g3, g4+g5] -> 16 instructions instead of 24.  CHUNKS lists
# (start ch, n ch, sem_dve value each DMA group in it waits for).
CHUNKS = [(0, 16), (16, 80), (96, 64), (160, 96)]
# chunk index whose completion each DMA group waits on
GROUP_CHUNK = [0, 1, 1, 2, 3, 3]
# (in0_offset_at_t0, in1_offset_at_t0) per r phase; t-stride is -2 for in0,
# +2 for in1.  All offsets/strides even (4B-aligned) -> DVE 2x_1p engages.
PHASES = [(128, 1024), (382, 1024), (382, 642), (126, 642)]

_nc_cache = None


def _sub(tile_ap, off, dims):
    """AP over `tile_ap`'s tensor: all 128 partitions, custom free dims."""
    if not isinstance(tile_ap, AP):
        tile_ap = tile_ap[:]
    part = list(tile_ap.ap)[0]
    return AP(
        tile_ap.tensor,
        tile_ap.offset + off,
        [list(part)] + [list(d) for d in dims],
    )


def build_nc():
    """Raw Bass (no TileContext): explicit sems, <=1 sync-wait per
    instruction (this walrus codegen path rejects multi-wait instructions,
    including Tile's tail drain)."""
    nc = bass.Bass(trn_type="TRN2")
    xin = nc.dram_tensor("xin", [P, XW], F16, kind="ExternalInput")
    out = nc.dram_tensor("out", [P, W * WH], F16, kind="ExternalOutput")

    from contextlib import ExitStack

    with ExitStack() as ctx:
        Xin = ctx.enter_context(nc.sbuf_tensor("Xin", [P, XW], F16))
        O = ctx.enter_context(nc.sbuf_tensor("O", [P, W * WH], F16))
        sem_in = ctx.enter_context(nc.semaphore("sem_in"))
        sem_dve = ctx.enter_context(nc.semaphore("sem_dve"))
        sem_out = ctx.enter_context(nc.semaphore("sem_out"))

        # SP: load the packed input (A/A1/B/B1 segments, host-staged fp16).
        nc.sync.dma_start(out=Xin[:], in_=xin[:]).then_inc(sem_in, 16)

        nc.vector.wait_ge(sem_in, 16)
        for ch0, n in CHUNKS:
            t0 = ch0 // 4
            nt = n // 4
            last = None
            for r, (i0, i1) in enumerate(PHASES):
                in0 = _sub(Xin, i0 - 2 * t0, [(-2, nt), (1, WH)])
                in1 = _sub(Xin, i1 + 2 * t0, [(2, nt), (1, WH)])
                o = _sub(O, (4 * t0 + r) * WH, [(4 * WH, nt), (1, WH)])
                last = nc.vector.tensor_tensor(
                    o, in0, in1, mybir.AluOpType.add
                )
            last.then_inc(sem_dve, 1)

        # Out DMAs alternate between the two HWDGE rings (SP / ACT); each
        # waits on the DVE chunk counter (1 wait per instruction).
        c0 = 0
        for g, n in enumerate(GROUPS):
            eng = nc.sync if g % 2 == 0 else nc.scalar
            eng.wait_ge(sem_dve, GROUP_CHUNK[g] + 1)
            eng.dma_start(
                out=out[:, c0 * WH:(c0 + n) * WH],
                in_=O[:, c0 * WH:(c0 + n) * WH],
            ).then_inc(sem_out, 16)
            c0 += n

        # Each issuing engine waits for all out-DMA completions so the
        # NEFF doesn't finish with DMAs in flight.
        nc.sync.wait_ge(sem_out, 16 * len(GROUPS))
        nc.scalar.wait_ge(sem_out, 16 * len(GROUPS))
    return nc


def get_nc():
    global _nc_cache
    if _nc_cache is None:
        _nc_cache = build_nc()
    return _nc_cache


def prep_in_maps(x, w1, w2):
    """Shard + stage inputs for the 8 cores (host-side data movement only).

    Packed per-core input [128, 1280] fp16, segments (element offsets):
      A @0    len 256: w1*x, padded so A[j] = w1*x[j-128] (half 0) / w1*x[j] (half 1)
      A1@256  len 256: A shifted one element left (A1[j] = A[j+1])
      B @512  len 384: w2*x, same padding as A
      B1@896  len 384: B shifted one element left
    The duplicated segments exist so every DVE access starts 4B-aligned
    (2x_1p requires it); odd logical shifts read the shifted copy instead.
    """
    x2 = np.ascontiguousarray(np.asarray(x, dtype=np.float32)[:, 0]).reshape(
        ROWS, W
    )
    w1v = np.float32(np.asarray(w1).reshape(-1)[0])
    w2v = np.float32(np.asarray(w2).reshape(-1)[0])
    in_maps = []
    for c in range(NCORES):
        rows = x2[c * RPC:(c + 1) * RPC]          # [64, 256] f32
        a = (w1v * rows).astype(np.float16)
        b = (w2v * rows).astype(np.float16)
        xin = np.zeros((P, XW), dtype=np.float16)
        # half 0 (partitions 0..63): output columns w in [0, 128)
        xin[:RPC, 128:256] = a[:, 0:128]          # A[j]  = w1*x[j-128]
        xin[:RPC, 383:511] = a[:, 0:128]          # A1[j] = w1*x[j-127]
        xin[:RPC, 640:896] = b                    # B[j]  = w2*x[j-128]
        xin[:RPC, 1023:1279] = b                  # B1[j] = w2*x[j-127]
        # half 1 (partitions 64..127): w in [128, 256)
        xin[RPC:, 0:256] = a                      # A[j]  = w1*x[j]
        xin[RPC:, 256:511] = a[:, 1:256]          # A1[j] = w1*x[j+1]
        xin[RPC:, 512:768] = b                    # B[j]  = w2*x[j]
        xin[RPC:, 896:1151] = b[:, 1:256]         # B1[j] = w2*x[j+1]
        in_maps.append({"xin": xin})
    return in_maps


def gather(outs):
    """Reassemble per-core [128, 256*128] fp16 outputs into [2,256,256,256] f32."""
    parts = []
    for oc in outs:
        oc = np.asarray(oc).reshape(2, RPC, W, WH)  # [whalf, row, ch, w']
        parts.append(oc.transpose(1, 2, 0, 3).reshape(RPC, W, W))
    out_rows = np.concatenate(parts, axis=0)        # [512 rows, ch, w]
    return np.ascontiguousarray(
        out_rows.reshape(2, 256, W, W).transpose(0, 2, 1, 3)
    ).astype(np.float32)


def kernel(x, w1, w2, _run_kwargs=None):
    from concourse.bass_utils import run_bass_kernel_spmd

    nc = get_nc()
    in_maps = prep_in_maps(x, w1, w2)
    kwargs = _run_kwargs or {}
    res = run_bass_kernel_spmd(nc, in_maps, core_ids=list(range(NCORES)), **kwargs)
    out = gather([r["out"] for r in res.results])
    if kwargs:
        kernel.last_results = res
    return out
